# revision 8
# baseline (speedup 1.0000x reference)
"""LIF (leaky integrate-and-fire) forward scan on 8 Trainium2 NeuronCores.

Reference recurrence (per element, scan over T):
    m_t = v_{t-1} * tau + x_t
    y_t = (m_t - v_th > 0) ? 1.0 : 0.0
    v_t = m_t * (1 - y_t)          # hard reset on spike

x: [T=16, B=32, C=128, H=32, W=32] f32.  Data-parallel over B: each core
gets B_loc=4 batches. Host pre-transposes the per-core block to
[T, C, F=4*H*W] so every per-step DMA is one fully-contiguous
16KiB-per-partition transfer.

Per step (F=4096 sites, one chunk):
  DVE: m  = scalar_tensor_tensor(v, tau, x, mult, add)     (skipped at t=0)
       v' = scalar_tensor_tensor(m, v_th, m, is_le, mult)  (skipped at t=T-1)
  ACT: y  = Sign(m - v_th) -> uint8 in ONE op (f32->u8 saturating convert
       maps -1 -> 0, so the result is exactly (m > v_th)); host -> f32.
  DMA: x load on the sync HWDGE ring, y store on the scalar ring.
DVE is the bottleneck at ~133 us busy; everything else overlaps.
"""

import sys

sys.path.insert(0, "/opt/trn_rl_repo")

from contextlib import ExitStack

import numpy as np

import concourse.bass as bass
import concourse.tile as tile
from concourse import bacc, mybir
from concourse.bass_utils import run_bass_kernel_spmd

# Hyperparameters (from the nn.Module)
V_TH = 1.0
TAU = 0.5

# Shapes (hardcoded per problem spec)
T, B, C, H, W = 16, 32, 128, 32, 32
N_CORES = 8
B_LOC = B // N_CORES           # 4 batches per core
S = H * W                      # 1024 spatial sites
F = B_LOC * S                  # 4096 free-dim sites per step

DT = mybir.dt.float32
U8 = mybir.dt.uint8


def build_kernel() -> bass.Bass:
    nc = bacc.Bacc(
        "TRN2", target_bir_lowering=False, debug=False, num_devices=N_CORES
    )
    x_d = nc.dram_tensor("x", [T, C, F], DT, kind="ExternalInput").ap()
    # Tiny [128,1] = -V_TH input: the ACT bias operand, DMA-loaded so no
    # engine memset + all_engine_barrier delays the pipeline start.
    b_d = nc.dram_tensor("bneg", [C, 1], DT, kind="ExternalInput").ap()
    y_d = nc.dram_tensor("y", [T, C, F], U8, kind="ExternalOutput").ap()

    with ExitStack() as ctx:
        tc = ctx.enter_context(tile.TileContext(nc))
        c_pool = ctx.enter_context(tc.tile_pool(name="c", bufs=1))
        x_pool = ctx.enter_context(tc.tile_pool(name="x", bufs=5))
        m_pool = ctx.enter_context(tc.tile_pool(name="m", bufs=3))
        v_pool = ctx.enter_context(tc.tile_pool(name="v", bufs=2))
        y_pool = ctx.enter_context(tc.tile_pool(name="y", bufs=3))

        bneg = c_pool.tile([C, 1], DT, tag="bneg")
        nc.scalar.dma_start(out=bneg[:], in_=b_d)

        v = None
        for t in range(T):
            xt = x_pool.tile([C, F], DT, tag="x")
            if t == 0:
                # Startup: 4 sub-loads split over both HWDGE rings so the
                # first compute starts ~1/4 load-time in.
                q = F // 4
                for i in range(4):
                    eng = nc.sync if i % 2 == 0 else nc.scalar
                    eng.dma_start(
                        out=xt[:, i * q:(i + 1) * q], in_=x_d[t, :, i * q:(i + 1) * q]
                    )
            else:
                nc.sync.dma_start(out=xt[:], in_=x_d[t])

            if t == 0:
                # m_0 = x_0; chunk the first step so compute ramps with the
                # sub-loads. v'_0 slices land in one [C, F] tile.
                yt = y_pool.tile([C, F], U8, tag="y")
                vt = v_pool.tile([C, F], DT, tag="v")
                q = F // 4
                for i in range(4):
                    sl = slice(i * q, (i + 1) * q)
                    nc.vector.scalar_tensor_tensor(
                        vt[:, sl], xt[:, sl], V_TH, xt[:, sl],
                        mybir.AluOpType.is_le, mybir.AluOpType.mult,
                    )
                    nc.scalar.activation(
                        yt[:, sl], xt[:, sl],
                        mybir.ActivationFunctionType.Sign, bias=bneg[:],
                    )
                nc.scalar.dma_start(out=y_d[t], in_=yt[:])
                v = vt
                continue

            if t == T - 1:
                # Tail: 2 sub-chunks so the last y/store overlaps the last STT.
                yt = y_pool.tile([C, F], U8, tag="y")
                mt = m_pool.tile([C, F], DT, tag="m")
                h = F // 2
                for i in range(2):
                    sl = slice(i * h, (i + 1) * h)
                    nc.vector.scalar_tensor_tensor(
                        mt[:, sl], v[:, sl], TAU, xt[:, sl],
                        mybir.AluOpType.mult, mybir.AluOpType.add,
                    )
                    nc.scalar.activation(
                        yt[:, sl], mt[:, sl],
                        mybir.ActivationFunctionType.Sign, bias=bneg[:],
                    )
                    nc.scalar.dma_start(out=y_d[t, :, sl], in_=yt[:, sl])
                continue

            mt = m_pool.tile([C, F], DT, tag="m")
            nc.vector.scalar_tensor_tensor(
                mt[:], v[:], TAU, xt[:],
                mybir.AluOpType.mult, mybir.AluOpType.add,
            )
            m = mt[:]

            yt = y_pool.tile([C, F], U8, tag="y")
            nc.scalar.activation(
                yt[:], m, mybir.ActivationFunctionType.Sign, bias=bneg[:]
            )
            nc.scalar.dma_start(out=y_d[t], in_=yt[:])

            vt = v_pool.tile([C, F], DT, tag="v")
            nc.vector.scalar_tensor_tensor(
                vt[:], m, V_TH, m,
                mybir.AluOpType.is_le, mybir.AluOpType.mult,
            )
            v = vt
    nc.finalize()
    return nc


_NC_CACHE = None


def _get_nc():
    global _NC_CACHE
    if _NC_CACHE is None:
        _NC_CACHE = build_kernel()
    return _NC_CACHE


def _in_maps(x: np.ndarray) -> list[dict]:
    xf = np.asarray(x, dtype=np.float32).reshape(T, B, C, S)
    maps = []
    for k in range(N_CORES):
        blk = xf[:, k * B_LOC:(k + 1) * B_LOC]          # [T, B_loc, C, S]
        blk = np.ascontiguousarray(blk.transpose(0, 2, 1, 3))  # [T, C, B_loc, S]
        maps.append({
            "x": blk.reshape(T, C, F),
            "bneg": np.full((C, 1), -V_TH, dtype=np.float32),
        })
    return maps


def kernel(x: np.ndarray) -> np.ndarray:
    assert x.shape == (T, B, C, H, W), x.shape
    in_dtype = x.dtype
    nc = _get_nc()
    in_maps = _in_maps(x)
    res = run_bass_kernel_spmd(nc, in_maps, list(range(N_CORES)))
    parts = []
    for k in range(N_CORES):
        yk = res.results[k]["y"].reshape(T, C, B_LOC, S).transpose(0, 2, 1, 3)
        parts.append(yk)                                # [T, B_loc, C, S]
    out = np.concatenate(parts, axis=1)                 # [T, B, C, S]
    return out.reshape(T, B, C, H, W).astype(in_dtype, copy=False)


if __name__ == "__main__":
    x = np.random.randn(T, B, C, H, W).astype(np.float32)
    y = kernel(x)
    print("out", y.shape, y.dtype, "spike rate", y.mean())


# revision 10
# speedup vs baseline: 1.0856x; 1.0856x over previous
"""LIF (leaky integrate-and-fire) forward scan on 8 Trainium2 NeuronCores.

Reference recurrence (per element, scan over T):
    m_t = v_{t-1} * tau + x_t
    y_t = (m_t - v_th > 0) ? 1.0 : 0.0
    v_t = m_t * (1 - y_t)          # hard reset on spike

x: [T=16, B=32, C=128, H=32, W=32] f32.  Data-parallel over B: each core
gets B_loc=4 batches. Host pre-transposes the per-core block to
[T, C, F=4*H*W] so every per-step DMA is one fully-contiguous
16KiB-per-partition transfer.

Per step (F=4096 sites, one chunk):
  DVE: m  = scalar_tensor_tensor(v, tau, x, mult, add)     (skipped at t=0)
       v' = scalar_tensor_tensor(m, v_th, m, is_le, mult)  (skipped at t=T-1)
  ACT: y  = Sign(m - v_th) -> uint8 in ONE op (f32->u8 saturating convert
       maps -1 -> 0, so the result is exactly (m > v_th)); host -> f32.
  DMA: x load on the sync HWDGE ring, y store on the scalar ring.
DVE is the bottleneck at ~133 us busy; everything else overlaps.
"""

import sys

sys.path.insert(0, "/opt/trn_rl_repo")

from contextlib import ExitStack

import numpy as np

import concourse.bass as bass
import concourse.tile as tile
from concourse import bacc, mybir
from concourse.bass_utils import run_bass_kernel_spmd

# Hyperparameters (from the nn.Module)
V_TH = 1.0
TAU = 0.5

# Shapes (hardcoded per problem spec)
T, B, C, H, W = 16, 32, 128, 32, 32
N_CORES = 8
B_LOC = B // N_CORES           # 4 batches per core
S = H * W                      # 1024 spatial sites
F = B_LOC * S                  # 4096 free-dim sites per step

DT = mybir.dt.float32
U8 = mybir.dt.uint8


def build_kernel() -> bass.Bass:
    nc = bacc.Bacc(
        "TRN2", target_bir_lowering=False, debug=False, num_devices=N_CORES
    )
    x_d = nc.dram_tensor("x", [T, C, F], DT, kind="ExternalInput").ap()
    y_d = nc.dram_tensor("y", [T, C, F], U8, kind="ExternalOutput").ap()

    # Register a -V_TH const AP (activation bias needs a [128,1] SBUF const).
    _c = nc.alloc_sbuf_tensor(f"const-float32-{-V_TH}", [128, 1], DT)
    nc.gpsimd.memset(_c.ap(), -V_TH)
    nc.const_aps.aps[(DT, -V_TH)] = _c.ap()
    nc.all_engine_barrier()

    with ExitStack() as ctx:
        tc = ctx.enter_context(tile.TileContext(nc))
        x_pool = ctx.enter_context(tc.tile_pool(name="x", bufs=5))
        m_pool = ctx.enter_context(tc.tile_pool(name="m", bufs=3))
        v_pool = ctx.enter_context(tc.tile_pool(name="v", bufs=2))
        y_pool = ctx.enter_context(tc.tile_pool(name="y", bufs=3))

        CH = 2
        W2 = F // CH
        v = [None] * CH
        for t in range(T):
            xt = x_pool.tile([C, F], DT, tag="x")
            for c in range(CH):
                sl = slice(c * W2, (c + 1) * W2)
                # At t=0 the scalar ring is still free: load halves in
                # parallel on both HWDGE rings for a faster ramp.
                eng = nc.scalar if (t == 0 and c == 1) else nc.sync
                eng.dma_start(out=xt[:, sl], in_=x_d[t, :, sl])

            # m for both chunks first, then y / v' — keeps the DVE queue
            # (m0, m1, v0, v1) stall-free across steps.
            ms = []
            for c in range(CH):
                sl = slice(c * W2, (c + 1) * W2)
                if t == 0:
                    ms.append(xt[:, sl])
                else:
                    mt = m_pool.tile([C, W2], DT, tag=f"m{c}", name=f"m{c}")
                    nc.vector.scalar_tensor_tensor(
                        mt[:], v[c][:], TAU, xt[:, sl],
                        mybir.AluOpType.mult, mybir.AluOpType.add,
                    )
                    ms.append(mt[:])

            yt = y_pool.tile([C, F], U8, tag="y")
            for c in range(CH):
                sl = slice(c * W2, (c + 1) * W2)
                nc.scalar.activation(
                    yt[:, sl], ms[c], mybir.ActivationFunctionType.Sign,
                    bias=-V_TH,
                )
                if t < T - 1:
                    vt = v_pool.tile([C, W2], DT, tag=f"v{c}", name=f"v{c}")
                    nc.vector.scalar_tensor_tensor(
                        vt[:], ms[c], V_TH, ms[c],
                        mybir.AluOpType.is_le, mybir.AluOpType.mult,
                    )
                    v[c] = vt
            nc.scalar.dma_start(out=y_d[t], in_=yt[:])
    nc.finalize()
    return nc


_NC_CACHE = None


def _get_nc():
    global _NC_CACHE
    if _NC_CACHE is None:
        _NC_CACHE = build_kernel()
    return _NC_CACHE


def _in_maps(x: np.ndarray) -> list[dict]:
    xf = np.asarray(x, dtype=np.float32).reshape(T, B, C, S)
    maps = []
    for k in range(N_CORES):
        blk = xf[:, k * B_LOC:(k + 1) * B_LOC]          # [T, B_loc, C, S]
        blk = np.ascontiguousarray(blk.transpose(0, 2, 1, 3))  # [T, C, B_loc, S]
        maps.append({"x": blk.reshape(T, C, F)})
    return maps


def kernel(x: np.ndarray) -> np.ndarray:
    assert x.shape == (T, B, C, H, W), x.shape
    in_dtype = x.dtype
    nc = _get_nc()
    in_maps = _in_maps(x)
    res = run_bass_kernel_spmd(nc, in_maps, list(range(N_CORES)))
    parts = []
    for k in range(N_CORES):
        yk = res.results[k]["y"].reshape(T, C, B_LOC, S).transpose(0, 2, 1, 3)
        parts.append(yk)                                # [T, B_loc, C, S]
    out = np.concatenate(parts, axis=1)                 # [T, B, C, S]
    return out.reshape(T, B, C, H, W).astype(in_dtype, copy=False)


if __name__ == "__main__":
    x = np.random.randn(T, B, C, H, W).astype(np.float32)
    y = kernel(x)
    print("out", y.shape, y.dtype, "spike rate", y.mean())
